# revision 43
# baseline (speedup 1.0000x reference)
"""Trainium2 Bass kernel for CellPathwayAttentionAggregator (segment-reduce).

Math: out[b, s] = sum_{i in set s} softmax_s(attn_logits)[i] * G[b, flat_idx[i]]

Device decomposition (per core, transposed output):
    out^T = (W_q^T @ G^T) * (1 / denom)[:, None]
where W_q[g, s] = fp8e3m4(exp(attn_logits[i] - max_s + ln 8)) for i in set s
with flat_idx[i] = g is the per-set max-normalized sparse aggregation matrix,
scattered on the host as pure layout prep (elementwise exp + scatter), and
    denom[s] = sum_{i in set s} w_q[i]
is computed ON DEVICE from padded ln(w_q) columns (ACT exp -> DVE free-axis
reduce -> DVE reciprocal). Sending ln(w_q) instead of raw logits makes the
device denominator EXACTLY consistent with the quantized numerator weights,
so the per-set fp8 scale (and the x8) cancels in the ratio and the softmax
normalization absorbs most of the W quantization error (measured 8.3e-3
absmax-rel vs the 2e-2 budget).

Why fp8e3m4 for W: the PE ingests stationary bytes through the same SBUF
port as the moving operand, so LDWEIGHTS time is byte-bound. Halving the
stationary bytes (vs bf16) cuts ~64 cycles per (K-tile, set-subtile) step
and halves W's DMA/SBUF footprint. The moving operand G stays bf16 (fp8
moving would need DoubleRow with BOTH operands fp8; G in fp8 alone costs
4e-2 error - over budget).

This build uses raw Bass with hand-placed semaphores (no Tile/Bacc
event-semaphore preamble ~7us or exit butterfly ~8us). Sharding: 8 cores =
2 batch groups (512 rows) x 4 set groups (512 sets). Each core accumulates
(512 sets x 8192) @ (8192 x 512 batch) into 4 PSUM banks over 64 K-tiles,
with inputs streamed as 4-K-tile chunked DMAs (G on the Sync HWDGE ring, W
on the Scalar ring) and a dependency-free PE warmup against the HAM
clock-gate.
"""

import sys

if "/opt/trn_rl_repo" not in sys.path:
    sys.path.insert(0, "/opt/trn_rl_repo")

import ml_dtypes
import numpy as np

NUM_SETS = 2048
NUM_GENESETS = 8192
BATCH = 1024
N_CORES = 8
BG, SG = 2, 4  # batch groups x set groups (BG*SG == N_CORES)
B_C = BATCH // BG  # 512 batch rows per core
S_C = NUM_SETS // SG  # 512 sets per core
P = 128
K_TILES = NUM_GENESETS // P  # 64
SUBT = S_C // P  # 4 set-subtiles of 128 sets
# K-tiles per input DMA. Each chunk must be a DENSE region in DRAM: chunk
# DMAs sliced out of a flat [P, X] tensor (64KB partition pitch) corrupted
# the stream under tracing - the DMA's 16 progressive sub-completions do
# not guarantee full-chunk visibility for strided patterns. Small head
# chunks (separate dense dram tensors) hide the ~2us DMA cold-start so the
# PE stream begins sooner.
CHUNK_SIZES = [4] * 16
assert sum(CHUNK_SIZES) == K_TILES
N_CHUNKS = len(CHUNK_SIZES)
CHUNK_MAX = max(CHUNK_SIZES)
N_HEAD = 0  # chunks with their own dram tensor
BUFS = 4  # chunk double-buffers
PAD_SLOTS = 128  # >= MAX set size (120)
NEG_FILL = -87.0  # exp(-87) ~ 1.6e-38 ~ 0 in fp32
N_WARMUP = 64  # dependency-free N=1 matmuls against the HAM clock gate
N_CLOCKHOLD = 320  # post-stream dummies: keep HAM at 8/8 through the tail

_PROGRAM_CACHE = {}
LAST_RESULTS = None  # BassKernelResults of the most recent run (for profiling)


def _build_program():
    import concourse.mybir as mybir
    from concourse import bass
    from contextlib import ExitStack

    f32 = mybir.dt.float32
    bf16 = mybir.dt.bfloat16
    f8e3 = mybir.dt.float8e3

    GW = CHUNK_MAX * B_C  # 2048 bf16 cols per G chunk slot
    WW = CHUNK_MAX * SUBT * P  # 2048 fp8 cols per W chunk slot

    nc = bass.Bass(trn_type="TRN2")
    # head chunks as separate dense tensors; body as one [n, P, cols] tensor
    gt_head = [
        nc.dram_tensor(f"gt{i}", [P, CHUNK_SIZES[i] * B_C], bf16, kind="ExternalInput")
        for i in range(N_HEAD)
    ]
    wt_head = [
        nc.dram_tensor(
            f"wt{i}", [P, CHUNK_SIZES[i] * SUBT * P], f8e3, kind="ExternalInput"
        )
        for i in range(N_HEAD)
    ]
    n_body = N_CHUNKS - N_HEAD
    gt_d = nc.dram_tensor("gt", [n_body, P, GW], bf16, kind="ExternalInput")
    wt_d = nc.dram_tensor("wt", [n_body, P, WW], f8e3, kind="ExternalInput")
    plog_d = nc.dram_tensor("plog", [P, SUBT * PAD_SLOTS], f32, kind="ExternalInput")
    out_d = nc.dram_tensor("out", [S_C, B_C], bf16, kind="ExternalOutput")

    with ExitStack() as ctx:
        g_sb = ctx.enter_context(nc.sbuf_tensor([P, BUFS, GW], bf16))
        w_sb = ctx.enter_context(nc.sbuf_tensor([P, BUFS, WW], f8e3))
        # head chunks get dedicated exactly-sized tensors: their DMAs then
        # write fully-dense destinations (partial-slot writes into the pool
        # corrupt under the same progressive-completion hazard as strided
        # sources)
        gh_sb = [
            ctx.enter_context(
                nc.sbuf_tensor(f"gh{i}", [P, CHUNK_SIZES[i] * B_C], bf16)
            )
            for i in range(N_HEAD)
        ]
        wh_sb = [
            ctx.enter_context(
                nc.sbuf_tensor(f"wh{i}", [P, CHUNK_SIZES[i] * SUBT * P], f8e3)
            )
            for i in range(N_HEAD)
        ]
        plog_sb = ctx.enter_context(nc.sbuf_tensor([P, SUBT * PAD_SLOTS], f32))
        exp_sb = ctx.enter_context(nc.sbuf_tensor([P, SUBT * PAD_SLOTS], f32))
        den_sb = ctx.enter_context(nc.sbuf_tensor([P, SUBT], f32))
        lnden_sb = ctx.enter_context(nc.sbuf_tensor([P, SUBT], f32))
        recip_sb = ctx.enter_context(nc.sbuf_tensor([P, SUBT], f32))
        ones_cell = ctx.enter_context(nc.sbuf_tensor([1, 1], bf16))
        o_sb = ctx.enter_context(nc.sbuf_tensor([P, SUBT, B_C], bf16))
        acc_ps = ctx.enter_context(nc.psum_tensor([P, SUBT, B_C], f32))
        scr_ps = ctx.enter_context(nc.psum_tensor([1, 1], f32))
        s_g = ctx.enter_context(nc.semaphore(name="s_g"))
        s_w = ctx.enter_context(nc.semaphore(name="s_w"))
        s_plog = ctx.enter_context(nc.semaphore(name="s_plog"))
        s_init = ctx.enter_context(nc.semaphore(name="s_init"))
        s_mm = ctx.enter_context(nc.semaphore(name="s_mm"))
        s_recip = ctx.enter_context(nc.semaphore(name="s_recip"))
        s_acc = ctx.enter_context(nc.semaphore(name="s_acc"))
        s_out_v = ctx.enter_context(nc.semaphore(name="s_out_v"))
        s_out_a = ctx.enter_context(nc.semaphore(name="s_out_a"))
        s_done = ctx.enter_context(nc.semaphore(name="s_done"))
        block = ctx.enter_context(nc.Block())

        chunk_off = [sum(CHUNK_SIZES[:i]) for i in range(N_CHUNKS)]

        @block.sync
        def _(sync):
            for ci in range(N_CHUNKS):
                if ci < N_HEAD:
                    sync.dma_start(gh_sb[ci][:], gt_head[ci][:, :]).then_inc(s_g, 16)
                    continue
                if ci - N_HEAD >= BUFS:
                    sync.wait_ge(s_mm, ci - BUFS + 1)
                sync.dma_start(
                    g_sb[:, (ci - N_HEAD) % BUFS, :], gt_d[ci - N_HEAD, :, :]
                ).then_inc(s_g, 16)
            sync.wait_ge(s_out_v, 1)
            sync.dma_start(out_d[0:P, :], o_sb[:, 0, :]).then_inc(s_done, 16)
            sync.wait_ge(s_out_a, 1)
            sync.dma_start(out_d[P : 2 * P, :], o_sb[:, 1, :]).then_inc(s_done, 16)
            sync.wait_ge(s_done, 64)

        @block.scalar
        def _(scalar):
            for ci in range(N_CHUNKS):
                if ci < N_HEAD:
                    scalar.dma_start(wh_sb[ci][:], wt_head[ci][:, :]).then_inc(
                        s_w, 16
                    )
                    continue
                if ci - N_HEAD >= BUFS:
                    scalar.wait_ge(s_mm, ci - BUFS + 1)
                scalar.dma_start(
                    w_sb[:, (ci - N_HEAD) % BUFS, :], wt_d[ci - N_HEAD, :, :]
                ).then_inc(s_w, 16)
            scalar.dma_start(plog_sb[:], plog_d[:, :]).then_inc(s_plog, 16)
            scalar.wait_ge(s_plog, 16)
            # denominators entirely ON ACT: exp with accum_out gives the
            # per-partition free-axis sum fused into the activation, so no
            # cross-engine exp->reduce handoff exists at all (a DVE reduce
            # reading ACT's exp output raced its write pipe under tracing)
            for j in range(SUBT):
                scalar.activation(
                    exp_sb[:, j * PAD_SLOTS : (j + 1) * PAD_SLOTS],
                    plog_sb[:, j * PAD_SLOTS : (j + 1) * PAD_SLOTS],
                    mybir.ActivationFunctionType.Exp,
                    accum_out=den_sb[:, j : j + 1],
                )
            # reciprocal as exp(-ln(den)) on ACT: InstReciprocal and custom
            # DVE ops don't survive the raw-Bass walrus codegen path, and
            # this measures 5e-5 rel err on hw.
            scalar.drain()
            scalar.activation(
                lnden_sb[:], den_sb[:], mybir.ActivationFunctionType.Ln
            )
            # ACT pipelines back-to-back activations with NO same-engine RAW
            # interlock through SBUF (trace: the Exp below started 92ns
            # before the Ln's writes landed, corrupting scattered recip
            # cells). Drain the pipe between them.
            scalar.drain()
            scalar.activation(
                recip_sb[:],
                lnden_sb[:],
                mybir.ActivationFunctionType.Exp,
                bias=0.0,
                scale=-1.0,
            )
            scalar.drain().then_inc(s_recip, 1)
            scalar.wait_ge(s_acc, 1)
            for j in (1, 3):
                scalar.activation(
                    o_sb[:, j, :],
                    acc_ps[:, j, :],
                    mybir.ActivationFunctionType.Copy,
                    bias=0.0,
                    scale=recip_sb[:, j : j + 1],
                )
                scalar.drain().then_inc(s_out_a, 1)
            scalar.wait_ge(s_out_v, 2)
            scalar.dma_start(out_d[2 * P : 3 * P, :], o_sb[:, 2, :]).then_inc(
                s_done, 16
            )
            scalar.dma_start(out_d[3 * P : 4 * P, :], o_sb[:, 3, :]).then_inc(
                s_done, 16
            )

        @block.tensor
        def _(tensor):
            tensor.wait_ge(s_init, 1)
            for _ in range(N_WARMUP):
                tensor.matmul(
                    scr_ps[:], ones_cell[:], ones_cell[:], start=True, stop=True
                )
            for ci in range(N_CHUNKS):
                k0, cs = chunk_off[ci], CHUNK_SIZES[ci]
                tensor.wait_ge(s_g, 16 * (ci + 1))
                tensor.wait_ge(s_w, 16 * (ci + 1))
                if ci < N_HEAD:
                    w_src, g_src = wh_sb[ci], gh_sb[ci]
                else:
                    slot = (ci - N_HEAD) % BUFS
                    w_src, g_src = w_sb[:, slot, :], g_sb[:, slot, :]
                for ki in range(cs):
                    k = k0 + ki
                    for j in range(SUBT):
                        mm = tensor.matmul(
                            acc_ps[:, j, :],
                            w_src[:, (ki * SUBT + j) * P : (ki * SUBT + j + 1) * P],
                            g_src[:, ki * B_C : (ki + 1) * B_C],
                            start=(k == 0),
                            stop=(k == K_TILES - 1),
                        )
                # operands fully streamed at retire of the chunk's last
                # matmul -> safe to overwrite this SBUF chunk slot
                mm.then_inc(s_mm, 1)
            tensor.drain().then_inc(s_acc, 1)
            # keep the HAM clock-gate at 8/8 while DVE/ACT normalize and the
            # out DMAs drain; nothing waits on these
            for _ in range(N_CLOCKHOLD):
                tensor.matmul(
                    scr_ps[:], ones_cell[:], ones_cell[:], start=True, stop=True
                )

        @block.vector
        def _(vector):
            vector.memset(ones_cell[:], 1.0).then_inc(s_init, 1)
            vector.wait_ge(s_recip, 1)
            vector.wait_ge(s_acc, 1)
            for j in (0, 2):
                nc.vector.tensor_scalar_mul(
                    o_sb[:, j, :], acc_ps[:, j, :], recip_sb[:, j : j + 1]
                )
                vector.drain().then_inc(s_out_v, 1)

    nc.finalize()
    return nc


def _get_program():
    if "nc" not in _PROGRAM_CACHE:
        _PROGRAM_CACHE["nc"] = _build_program()
    return _PROGRAM_CACHE["nc"]


def _ensure_ntff_hook():
    """Make NTFF profiling under axon work (BASS_TRACE=1): the image's antenv
    package lacks the axon_hooks holder module, so synthesize it and register
    the ctypes-based profile hook from trn_agent_boot. Best-effort."""
    import types

    try:
        import antenv

        try:
            from antenv.axon_hooks import get_axon_ntff_profile_hook  # noqa: F401

            return  # already present and registered
        except ImportError:
            pass
        mod = types.ModuleType("antenv.axon_hooks")
        _holder = [None]
        mod.set_axon_ntff_profile_hook = lambda h: _holder.__setitem__(0, h)
        mod.get_axon_ntff_profile_hook = lambda: _holder[0]
        sys.modules["antenv.axon_hooks"] = mod
        antenv.axon_hooks = mod

        from trn_agent_boot.trn_boot import _ntff_profile_via_ctypes

        hook = _ntff_profile_via_ctypes("/opt/axon/libaxon_pjrt.so")
        mod.set_axon_ntff_profile_hook(hook)
    except Exception:
        pass


def kernel(**inputs):
    global LAST_RESULTS
    G = np.asarray(inputs["geneset_features"], dtype=np.float32)
    logits = np.asarray(inputs["attn_logits"], dtype=np.float32)
    flat_idx = np.asarray(inputs["flat_idx"]).astype(np.int64)
    seg = np.asarray(inputs["segment_ids"]).astype(np.int64)
    T = logits.shape[0]
    f8 = ml_dtypes.float8_e3m4
    bf16 = ml_dtypes.bfloat16

    # Host-side layout prep: per-set max-normalize (x8 to center the fp8e3m4
    # range, max 15.5), quantize to fp8, scatter into the sparse aggregation
    # matrix. Member sets are sampled without replacement, so (idx, seg)
    # pairs are unique within a set and the fancy assignment collision-free.
    segmax = np.full(NUM_SETS, -np.inf, dtype=np.float32)
    np.maximum.at(segmax, seg, logits)
    wq8 = np.exp(logits - segmax[seg] + np.log(8.0)).astype(f8)
    wqf = wq8.astype(np.float32)
    W8 = np.zeros((NUM_GENESETS, NUM_SETS), dtype=f8)
    W8[flat_idx, seg] = wq8

    # Padded per-set ln(w_q) columns; the device exp/reduce then reproduces
    # sum_i w_q exactly, keeping the denominator consistent with the
    # quantized numerator (fp8 scale cancels in the ratio).
    plog_vals = np.full(T, NEG_FILL, dtype=np.float32)
    m = wqf > 0
    plog_vals[m] = np.log(wqf[m])
    sizes = np.bincount(seg, minlength=NUM_SETS)
    starts = np.concatenate([[0], np.cumsum(sizes)[:-1]])
    pos = np.arange(T) - starts[seg]
    plogT = np.full((PAD_SLOTS, NUM_SETS), NEG_FILL, dtype=np.float32)
    plogT[pos, seg] = plog_vals

    GbT = np.ascontiguousarray(G.astype(bf16).T)  # (8192, 1024)
    in_maps = []
    for c in range(N_CORES):
        bg, sg = divmod(c, SG)
        gt = GbT[:, bg * B_C : (bg + 1) * B_C]  # (8192, 512)
        # per-K-tile partition-major blocks: gtk[k, p, b] = G^T[k*128 + p, b]
        gtk = gt.reshape(K_TILES, P, B_C)
        wt = W8[:, sg * S_C : (sg + 1) * S_C]  # (8192, 512)
        wtk = wt.reshape(K_TILES, P, SUBT * P)
        core_map = {}
        k0 = 0
        for i in range(N_HEAD):
            cs = CHUNK_SIZES[i]
            core_map[f"gt{i}"] = np.ascontiguousarray(
                gtk[k0 : k0 + cs].transpose(1, 0, 2).reshape(P, cs * B_C)
            )
            core_map[f"wt{i}"] = np.ascontiguousarray(
                wtk[k0 : k0 + cs].transpose(1, 0, 2).reshape(P, cs * SUBT * P)
            )
            k0 += cs
        n_body = N_CHUNKS - N_HEAD
        core_map["gt"] = np.ascontiguousarray(
            gtk[k0:]
            .reshape(n_body, CHUNK_MAX, P, B_C)
            .transpose(0, 2, 1, 3)
            .reshape(n_body, P, CHUNK_MAX * B_C)
        )
        core_map["wt"] = np.ascontiguousarray(
            wtk[k0:]
            .reshape(n_body, CHUNK_MAX, P, SUBT * P)
            .transpose(0, 2, 1, 3)
            .reshape(n_body, P, CHUNK_MAX * SUBT * P)
        )
        # sets-on-partitions layout: plog[s_local, j*128+t] = ln(w_q) slot t
        # of set (sg*S_C + j*128 + s_local)
        chunk = plogT[:, sg * S_C : (sg + 1) * S_C]  # (slots, S_C)
        core_map["plog"] = np.ascontiguousarray(
            chunk.reshape(PAD_SLOTS, SUBT, P).transpose(2, 1, 0).reshape(P, -1)
        )
        in_maps.append(core_map)

    from concourse.bass_utils import run_bass_kernel_spmd

    _ensure_ntff_hook()
    nc = _get_program()
    res = run_bass_kernel_spmd(nc, in_maps, core_ids=list(range(N_CORES)))
    LAST_RESULTS = res

    out = np.empty((BATCH, NUM_SETS), dtype=np.float32)
    for c in range(N_CORES):
        bg, sg = divmod(c, SG)
        out[bg * B_C : (bg + 1) * B_C, sg * S_C : (sg + 1) * S_C] = (
            res.results[c]["out"].astype(np.float32).T
        )
    return out
